# revision 3
# baseline (speedup 1.0000x reference)
"""Trainium2 Bass kernel for unscaled dot-product attention.

Shapes (hardcoded): query/key/value [2048, 2, 16, 64] fp32.
  scores = einsum('sbnh,tbnh->bnst', q, k)   (UNscaled)
  probs  = softmax(scores, axis=-1)
  out    = einsum('bnst,tbnh->sbnh', probs, v).reshape(2048, 2, 1024)

Sharding: the 32 (b, n) head-slices are split 4-per-core across 8 cores
(core c -> b = c//4, heads 4*(c%4) .. +4). Each core computes attention
for its 4 heads independently; no cross-device communication.

Per-core design (heads in 2 pairs; 128 steps of (pair, s-chunk, t-block)):
  - Q (pre-scaled by log2(e)) and K are fp16; scoresT per 128-t block are
    computed by TWO row-tiled K=64 matmuls (heads packed as PE tiles
    T0/T8) that co-stream, halving QK time vs fp32.
  - exp is computed as 2^t, alternating between TWO single-instruction
    engine pipelines (GPSIMD has no PSUM port, so only ACT/DVE qualify):
      * even steps -- Scalar engine: spline Exp with scale=ln2, bf16 out.
      * odd steps  -- Vector engine: ONE tensor_scalar op
        (t + B)*128 -> int16, whose bits are exactly the bf16 pattern of
        2^t under the linear-mantissa (Schraudolph) approximation.
        Max rel err ~3%, which softmax normalization largely cancels
        (measured ~5.5e-3 end-to-end vs the 2e-2 budget).
    Each lane runs one op per 2 steps (~1.1-1.2us) vs a 2-step PE budget
    of ~1.3us, so exp stays off the PE's critical path.
  - PV accumulates context^T in PSUM with bf16 V' (64 V cols + ones col
    producing the softmax denominator).  Heads go to separate 1-bank CT
    tiles so 3 score buffers + 2 CTs fit the 8 PSUM banks.
  - Normalization happens ON THE HOST: unnormalized CT + denominator row
    are copied out (ACT + DVE) and DMA-ed from the Sync queue; the host
    divides and transposes.
  - Input DMAs are spread across the Sync/GpSimd/Vector engine queues
    (separate hardware DMA queues -> parallel transfers) in consumption
    order, and ~36 dummy warm-up matmuls run during the DMA head so the
    PE's HAM clock gate releases before real compute starts.
  - PVs are flushed three at a time ([QK x3, PV x6] PE-queue blocks) to
    cut the 64-row/128-row tiling-mode switches on the tensor engine.
"""

import numpy as np

SQ, B, NHEADS, HN = 2048, 2, 16, 64
N_CORES = 8
HEADS_PER_CORE = 4
VW = 66                     # V' live rows per head (64 V + ones + pad)
LOG2E = 1.4426950408889634
LN2 = 0.6931471805599453

SCH = 512                   # s-chunk per inner loop
NCH = SQ // SCH             # 4
NT = SQ // 128              # 16 t-blocks

# Schraudolph exp2 bias: int16((t + B)*128) == bf16 bits of ~2^t.
# 127 - 0.043 centers the linear-mantissa log error; +0.5/128 centers
# the f32->int16 truncation (a round-mode instead just shifts the global
# scale, which softmax normalization cancels).
SCHRAU_B = 127.0 - 0.043 + 0.5 / 128

_CACHE = {}


def _build_program(lag=4, batch=3):
    from contextlib import ExitStack

    import concourse.bacc as bacc
    import concourse.mybir as mybir
    import concourse.tile as tile

    f32 = mybir.dt.float32
    bf16 = mybir.dt.bfloat16
    fp16 = mybir.dt.float16
    i16 = mybir.dt.int16
    EXP = mybir.ActivationFunctionType.Exp

    nc = bacc.Bacc("TRN2", target_bir_lowering=False, debug=False,
                   num_devices=N_CORES)

    kq = nc.dram_tensor("kq", [2, 2, 128, SQ], fp16,
                        kind="ExternalInput").ap()
    vv = nc.dram_tensor("vv", [2, 128, NT * 2 * VW], bf16,
                        kind="ExternalInput").ap()
    outU = nc.dram_tensor("outU", [2, VW, 2 * SQ], f32,
                          kind="ExternalOutput").ap()

    with tile.TileContext(nc) as tc, ExitStack() as ctx:
        in_pool = ctx.enter_context(tc.tile_pool(name="ins", bufs=1))
        ex_pool = ctx.enter_context(tc.tile_pool(name="ex", bufs=5))
        exi_pool = ctx.enter_context(tc.tile_pool(name="exi", bufs=5))
        cts_pool = ctx.enter_context(tc.tile_pool(name="cts", bufs=2))
        # PSUM: sc 3 bufs x 2 banks + ct 2 bufs x 1 bank = 8 banks
        ps_sc = ctx.enter_context(tc.tile_pool(name="ps_sc", bufs=3, space="PSUM"))
        ps_cta = ctx.enter_context(tc.tile_pool(name="ps_cta", bufs=1, space="PSUM"))
        ps_ctb = ctx.enter_context(tc.tile_pool(name="ps_ctb", bufs=1, space="PSUM"))

        # --- input tiles + ordered DMA ---------------------------------
        kt = [in_pool.tile([128, SQ], fp16, tag=f"kt{g}", name=f"kt{g}")
              for g in range(2)]
        qt = [in_pool.tile([128, SQ], fp16, tag=f"qt{g}", name=f"qt{g}")
              for g in range(2)]
        vt = [in_pool.tile([128, NT * 2 * VW], bf16, tag=f"vt{g}", name=f"vt{g}")
              for g in range(2)]
        VH = 8 * 2 * VW          # half of the V' columns (j-blocks 0-7)
        QH = 4 * 2 * VW          # V' columns for j-blocks 0-3
        # Consumption order, spread across the Sync/GpSimd/Scalar engine DMA queues so the
        # first chunks of K/Q/V stream in parallel.
        nc.sync.dma_start(out=kt[0][:, 0:512], in_=kq[0, 0, :, 0:512])
        nc.gpsimd.dma_start(out=qt[0][:, 0:1024], in_=kq[0, 1, :, 0:1024])
        nc.scalar.dma_start(out=vt[0][:, 0:QH], in_=vv[0, :, 0:QH])
        nc.sync.dma_start(out=kt[0][:, 512:1024], in_=kq[0, 0, :, 512:1024])
        nc.scalar.dma_start(out=vt[0][:, QH:VH], in_=vv[0, :, QH:VH])
        nc.sync.dma_start(out=kt[0][:, 1024:2048], in_=kq[0, 0, :, 1024:2048])
        nc.scalar.dma_start(out=vt[0][:, VH:2 * VH], in_=vv[0, :, VH:2 * VH])
        nc.gpsimd.dma_start(out=qt[0][:, 1024:2048], in_=kq[0, 1, :, 1024:2048])
        # pair 1
        nc.sync.dma_start(out=kt[1][:], in_=kq[1, 0])
        nc.gpsimd.dma_start(out=qt[1][:], in_=kq[1, 1])
        nc.scalar.dma_start(out=vt[1][:], in_=vv[1])

        v3 = [vt[g].rearrange("p (j c) -> p j c", c=2 * VW) for g in range(2)]

        # PE warm-up: ~4us of dummy matmuls with no DMA dependency, issued
        # while the input DMA streams in, so the HAM clock gate releases
        # before real compute starts.
        wz = in_pool.tile([128, 128], bf16, tag="wz", name="wz")
        nc.gpsimd.memset(wz[:], 0)
        wps = ps_sc.tile([128, 1024], f32, tag="sc", name="warm")
        for _ in range(36):
            nc.tensor.matmul(wps[:, 0:128], lhsT=wz[:], rhs=wz[:, 0:128],
                             start=True, stop=True)

        steps = [(g, c, j) for g in range(2) for c in range(NCH)
                 for j in range(NT)]

        def emit_qk(s):
            g, c, j = steps[s]
            s0 = c * SCH
            sc = ps_sc.tile([128, 1024], f32, tag="sc", name="sc")
            nc.tensor.matmul(
                sc[:, 0:512],
                lhsT=kt[g][0:64, j * 128:(j + 1) * 128],
                rhs=qt[g][0:64, s0:s0 + SCH],
                start=True, stop=True)
            nc.tensor.matmul(
                sc[:, 512:1024],
                lhsT=kt[g][64:128, j * 128:(j + 1) * 128],
                rhs=qt[g][64:128, s0:s0 + SCH],
                start=True, stop=True)
            return sc

        CT = {}

        def emit_pv(s, ex):
            g, c, j = steps[s]
            if j == 0:
                CT[0] = ps_cta.tile([128, 512], f32, tag="cta", name="cta")
                CT[1] = ps_ctb.tile([128, 512], f32, tag="ctb", name="ctb")
            nc.tensor.matmul(
                CT[0][0:VW, 0:512],
                lhsT=v3[g][:, j, 0:VW],
                rhs=ex[:, 0:512],
                start=(j == 0), stop=(j == NT - 1))
            nc.tensor.matmul(
                CT[1][0:VW, 0:512],
                lhsT=v3[g][:, j, VW:2 * VW],
                rhs=ex[:, 512:1024],
                start=(j == 0), stop=(j == NT - 1))

        def emit_tail(s):
            g, c, j = steps[s]
            if j != NT - 1:
                return
            s0 = c * SCH
            cta = cts_pool.tile([VW, 512], f32, tag="cta_s", name="cta_s")
            nc.scalar.copy(cta[:], CT[0][0:VW, :])
            ctb = cts_pool.tile([VW, 512], f32, tag="ctb_s", name="ctb_s")
            nc.vector.tensor_copy(ctb[:], CT[1][0:VW, :])
            nc.sync.dma_start(out=outU[g, :, s0:s0 + SCH],
                              in_=cta[:])
            nc.sync.dma_start(out=outU[g, :, SQ + s0:SQ + s0 + SCH],
                              in_=ctb[:])

        def emit_exp(s, sc):
            if s % 2 == 0:
                ex = ex_pool.tile([128, 1024], bf16, tag="ex", name="ex")
                nc.scalar.activation(ex[:], sc[:], EXP, scale=LN2)
                return ex
            exi = exi_pool.tile([128, 1024], i16, tag="exi", name="exi")
            nc.vector.tensor_scalar(
                exi[:], sc[:], SCHRAU_B, 128.0,
                op0=mybir.AluOpType.add, op1=mybir.AluOpType.mult)
            return exi.bitcast(bf16)

        # Software pipeline: exp chases QK immediately; PV trails by `lag`
        # steps so exp-lane latency stays off the PE's in-order critical
        # path.  PVs flush `batch` at a time so the PE queue alternates
        # [QK x batch, PV x 2*batch] blocks, cutting tiling-mode switches.
        pvq = []
        sc_cur = emit_qk(0)
        for s in range(len(steps) + lag):
            if s < len(steps):
                sc_next = emit_qk(s + 1) if s + 1 < len(steps) else None
                pvq.append((s, emit_exp(s, sc_cur)))
                sc_cur = sc_next
            if s >= lag and (s - lag) % batch == batch - 1:
                for _ in range(batch):
                    sp, exp_tile = pvq.pop(0)
                    emit_pv(sp, exp_tile)
                    emit_tail(sp)
        while pvq:
            sp, exp_tile = pvq.pop(0)
            emit_pv(sp, exp_tile)
            emit_tail(sp)
    nc.compile()
    return nc


def get_nc(lag=4, batch=3):
    key = ("nc", lag, batch)
    if key not in _CACHE:
        _CACHE[key] = _build_program(lag, batch)
    return _CACHE[key]


def make_in_maps(query, key, value):
    """Host-side sharding + layout prep. Returns list of per-core input maps."""
    query = np.asarray(query, dtype=np.float32) * np.float32(LOG2E)
    key = np.asarray(key, dtype=np.float32)
    value = np.asarray(value, dtype=np.float32)
    in_maps = []
    for c in range(N_CORES):
        b = c // 4
        n0 = HEADS_PER_CORE * (c % 4)
        q = query[:, b, n0:n0 + 4, :]   # [2048, 4, 64]
        k = key[:, b, n0:n0 + 4, :]
        v = value[:, b, n0:n0 + 4, :]
        qt = q.transpose(1, 2, 0).reshape(2, 128, SQ).astype(np.float16)
        kt = k.transpose(1, 2, 0).reshape(2, 128, SQ).astype(np.float16)
        kq = np.ascontiguousarray(np.stack([kt, qt], axis=1))  # [2,2,128,SQ]
        vp = np.concatenate(
            [v, np.ones((SQ, 4, 1), np.float32),
             np.zeros((SQ, 4, 1), np.float32)], axis=2)
        vp = vp.reshape(16, 128, 2, 2 * VW).transpose(2, 1, 0, 3)
        import ml_dtypes
        vp = np.ascontiguousarray(
            vp.reshape(2, 128, NT * 2 * VW)).astype(ml_dtypes.bfloat16)
        in_maps.append({"kq": kq, "vv": vp})
    return in_maps


def postprocess_core(outU):
    """outU [2, 66, 4096] -> normalized per-core output [2048, 4, 64]."""
    outU = np.asarray(outU)
    res = np.empty((SQ, 4, HN), np.float32)
    for g in range(2):
        for h in range(2):
            blk = outU[g, :, h * SQ:(h + 1) * SQ]
            ctx = blk[0:64, :]
            den = blk[64, :]
            res[:, 2 * g + h, :] = (ctx / den).T
    return res


def assemble_output(results):
    out = np.empty((SQ, B, NHEADS, HN), np.float32)
    for c in range(N_CORES):
        b = c // 4
        n0 = HEADS_PER_CORE * (c % 4)
        out[:, b, n0:n0 + 4, :] = postprocess_core(results[c]["outU"])
    return out.reshape(SQ, B, NHEADS * HN)


def kernel(query, key, value):
    try:
        from concourse.bass_utils import run_bass_kernel_spmd
    except ImportError:
        import sys
        sys.path.insert(0, "/opt/trn_rl_repo")
        from concourse.bass_utils import run_bass_kernel_spmd

    nc = get_nc()
    in_maps = make_in_maps(query, key, value)
    res = run_bass_kernel_spmd(nc, in_maps, list(range(N_CORES)))
    return assemble_output(res.results)
